# revision 1
# baseline (speedup 1.0000x reference)
"""GCN encoder (3x GCNConv + mean-pool + MLP) as an 8-core Trainium2 Bass kernel.

Sharding: nodes/edges partitioned by destination-node owner (8 shards).
Per layer: per-edge source features are gathered from a per-core DRAM table
(fp16) with dma_gather, scaled+scattered into per-destination sums via a
PE matmul against a one-hot selection matrix built on DVE, then the layer
weight matmul + bias + ReLU produces this core's shard of the next layer's
features, which an AllGather collective replicates into every core's table.
Mean-pool is a matmul against a per-graph one-hot (scaled by 1/count),
AllReduce-summed across cores; the tiny MLP is computed replicated.
"""

import numpy as np

NCORES = 8
F = 128            # hidden width (all layers padded to this)
G = 256            # number of graphs
NH = 512           # MLP hidden
NO = 256           # MLP out
CH = 128           # edges per chunk
BATCH_CH = 32      # chunks per dma_gather batch
WINW = 256         # dst nodes per PSUM accumulation window

_cache = {}


def _host_prep(x, edge_index, batch, W0, b0, W1, b1, W2, b2, Wm1, bm1, Wm2, bm2):
    N = x.shape[0]
    FI = x.shape[1]
    SH = -(-N // (NCORES * 128)) * 128      # shard size (nodes), 128-multiple
    NP = SH * NCORES
    TILES = SH // 128
    NWIN = -(-SH // WINW)
    LO = min(32768, NP)
    HI = NP - LO

    src = np.concatenate([edge_index[0], np.arange(N, dtype=np.int64)])
    dst = np.concatenate([edge_index[1], np.arange(N, dtype=np.int64)])
    deg = np.bincount(dst, minlength=N).astype(np.float32)
    dis = np.where(deg > 0, 1.0 / np.sqrt(np.maximum(deg, 1.0)), 0.0).astype(np.float32)
    norm = dis[src] * dis[dst]

    xpad = np.zeros((NP, F), dtype=np.float16)
    xpad[:N, :FI] = x.astype(np.float16)

    # per-core edge selection, ordered by (window, class, dst)
    per_core = []
    for c in range(NCORES):
        base = c * SH
        sel = (dst >= base) & (dst < base + SH)
        es = src[sel].astype(np.int64)
        ed = (dst[sel] - base).astype(np.int64)
        en = norm[sel]
        cl = (es >= LO).astype(np.int64)
        wi = ed // WINW
        order = np.lexsort((ed, cl, wi))
        per_core.append((es[order], ed[order], en[order], cl[order], wi[order]))

    # chunk counts per (window, class), equalized across cores
    nch = np.zeros((NWIN, 2), dtype=np.int64)
    counts = np.zeros((NCORES, NWIN, 2), dtype=np.int64)
    for c in range(NCORES):
        _, _, _, cl, wi = per_core[c]
        for cls in (0, 1):
            cnt = np.bincount(wi[cl == cls], minlength=NWIN)
            counts[c, :, cls] = cnt
    nch = -(-counts.max(axis=0) // CH)  # [NWIN, 2] chunks
    nch_cls = nch.sum(axis=0)          # total chunks per class
    ncht = int(nch.sum())

    # shared program schedule: windows -> list of (cls, cid); meta col = global g
    schedule = []
    cid_ctr = [0, 0]
    for w in range(NWIN):
        lst = []
        for cls in (0, 1):
            for _ in range(int(nch[w, cls])):
                lst.append((cls, cid_ctr[cls]))
                cid_ctr[cls] += 1
        schedule.append(lst)

    # per-core streams
    idx_streams = [[], []]   # per class: list over cores of int16 arrays
    metas = []
    for c in range(NCORES):
        es, ed, en, cl, wi = per_core[c]
        idx_parts = [[], []]
        meta = np.zeros((128, 2 * ncht), dtype=np.float32)
        g = 0
        pos = 0
        # edges are sorted (win, cls, dst); walk groups in the same order
        for w in range(NWIN):
            for cls in (0, 1):
                n_e = int(counts[c, w, cls])
                tot = int(nch[w, cls]) * CH
                ge, gd, gn = es[pos:pos + n_e], ed[pos:pos + n_e], en[pos:pos + n_e]
                pos += n_e
                pad = tot - n_e
                iv = ge - (LO if cls else 0)
                iv = np.concatenate([iv, np.zeros(pad, np.int64)])
                dl = np.concatenate([gd - w * WINW, np.zeros(pad, np.int64)])
                nr = np.concatenate([gn, np.zeros(pad, np.float32)])
                idx_parts[cls].append(iv.astype(np.int16))
                for k in range(tot // CH):
                    meta[:, 2 * g] = dl[k * CH:(k + 1) * CH].astype(np.float32)
                    meta[:, 2 * g + 1] = nr[k * CH:(k + 1) * CH].astype(np.float32)
                    g += 1
        assert g == ncht
        for cls in (0, 1):
            arr = (np.concatenate(idx_parts[cls]) if idx_parts[cls]
                   else np.zeros(0, np.int16))
            assert arr.size == nch_cls[cls] * CH
            if arr.size:
                wrapped = np.tile(arr.reshape(-1, 16).T, (8, 1))
            else:
                wrapped = np.zeros((128, 8), np.int16)  # dummy
            idx_streams[cls].append(np.ascontiguousarray(wrapped))
        metas.append(meta)

    # pooling helpers
    cnt = np.bincount(batch.astype(np.int64), minlength=G).astype(np.float32)
    invc_all = (1.0 / np.maximum(cnt, 1.0))[batch.astype(np.int64)]
    bcols, invcs = [], []
    for c in range(NCORES):
        sl = slice(c * SH, min((c + 1) * SH, N))
        b_sh = np.zeros(SH, np.float32)
        i_sh = np.zeros(SH, np.float32)
        nreal = max(0, min((c + 1) * SH, N) - c * SH)
        if nreal > 0:
            b_sh[:nreal] = batch[sl].astype(np.float32)
            i_sh[:nreal] = invc_all[sl].astype(np.float32)
        bcols.append(np.ascontiguousarray(b_sh.reshape(TILES, 128).T))  # [128,TILES]
        invcs.append(np.ascontiguousarray(i_sh.reshape(TILES, 128).T))

    W0p = np.zeros((F, F), np.float16)
    W0p[:FI] = W0.astype(np.float16)
    consts = {
        "w0": W0p, "w1": W1.astype(np.float16), "w2": W2.astype(np.float16),
        "wm1": Wm1.astype(np.float16), "wm2": Wm2.astype(np.float16),
        "b0r": np.tile(b0.astype(np.float32)[None, :], (128, 1)),
        "b1r": np.tile(b1.astype(np.float32)[None, :], (128, 1)),
        "b2r": np.tile(b2.astype(np.float32)[None, :], (128, 1)),
        "bm1c": np.ascontiguousarray(bm1.astype(np.float32).reshape(4, 128).T),
        "bm2r": np.tile(bm2.astype(np.float32)[None, :], (128, 1)),
        "iota": np.tile(np.arange(G, dtype=np.float16)[None, :], (128, 1)),
    }
    in_maps = []
    for c in range(NCORES):
        m = dict(consts)
        m["xtab"] = xpad
        m["idxlo"] = idx_streams[0][c]
        m["idxhi"] = idx_streams[1][c]
        m["meta"] = metas[c]
        m["bcol"] = bcols[c]
        m["invc"] = invcs[c]
        in_maps.append(m)

    geom = dict(N=N, NP=NP, SH=SH, TILES=TILES, NWIN=NWIN, LO=LO, HI=HI,
                nch=nch, nch_cls=[int(v) for v in nch_cls], ncht=ncht,
                schedule=schedule)
    return geom, in_maps


class _SkipRest(Exception):
    pass


def _build_bass(geom, variant="full"):
    import concourse.bass as bass
    import concourse.tile as tile
    from concourse import bacc, mybir

    f16, f32, i16 = mybir.dt.float16, mybir.dt.float32, mybir.dt.int16
    NP, SH, TILES, NWIN = geom["NP"], geom["SH"], geom["TILES"], geom["NWIN"]
    LO, HI = geom["LO"], geom["HI"]
    nch, nch_cls, ncht = geom["nch"], geom["nch_cls"], geom["ncht"]
    schedule = geom["schedule"]

    nc = bacc.Bacc("TRN2", target_bir_lowering=False, debug=False,
                   num_devices=NCORES)

    xtab = nc.dram_tensor("xtab", [NP, F], f16, kind="ExternalInput")
    idxlo = nc.dram_tensor("idxlo", [128, max(nch_cls[0] * 8, 8)], i16, kind="ExternalInput")
    idxhi = nc.dram_tensor("idxhi", [128, max(nch_cls[1] * 8, 8)], i16, kind="ExternalInput")
    meta = nc.dram_tensor("meta", [128, 2 * ncht], f32, kind="ExternalInput")
    w_in = {n: nc.dram_tensor(n, [F, F], f16, kind="ExternalInput")
            for n in ("w0", "w1", "w2")}
    wm1 = nc.dram_tensor("wm1", [F, NH], f16, kind="ExternalInput")
    wm2 = nc.dram_tensor("wm2", [NH, NO], f16, kind="ExternalInput")
    b_in = {n: nc.dram_tensor(n, [128, F], f32, kind="ExternalInput")
            for n in ("b0r", "b1r", "b2r")}
    bm1c = nc.dram_tensor("bm1c", [128, 4], f32, kind="ExternalInput")
    bm2r = nc.dram_tensor("bm2r", [128, NO], f32, kind="ExternalInput")
    iota = nc.dram_tensor("iota", [128, G], f16, kind="ExternalInput")
    bcol = nc.dram_tensor("bcol", [128, TILES], f32, kind="ExternalInput")
    invc = nc.dram_tensor("invc", [128, TILES], f32, kind="ExternalInput")
    out = nc.dram_tensor("out", [G, NO], f32, kind="ExternalOutput")

    shard_d = nc.dram_tensor("shard_d", [SH, F], f16)
    tabn = nc.dram_tensor("tabn", [NP, F], f16, addr_space="Shared")
    gt_in = nc.dram_tensor("gt_in", [128, G], f32)
    gt_out = nc.dram_tensor("gt_out", [128, G], f32, addr_space="Shared")

    shb = nc.alloc_sbuf_tensor("shb", [128, TILES * F], f16)

    import contextlib
    with tile.TileContext(nc) as tc:
        with (
            contextlib.suppress(_SkipRest),
            tc.tile_pool(name="res", bufs=1) as res,
            tc.tile_pool(name="msg", bufs=3) as msgp,
            tc.tile_pool(name="sp", bufs=4) as sp,
            tc.tile_pool(name="agg", bufs=2) as aggp,
            tc.tile_pool(name="tmp", bufs=2) as tmpp,
            tc.tile_pool(name="wps", bufs=2, space="PSUM") as wps,
            tc.tile_pool(name="hps", bufs=2, space="PSUM") as hps,
            tc.tile_pool(name="gps", bufs=1, space="PSUM") as gps,
            tc.tile_pool(name="mps", bufs=1, space="PSUM") as mps,
        ):
            # ---- resident loads ----
            def load(t_dram, shape, dtype):
                t = res.tile(shape, dtype, tag=t_dram.name)
                nc.sync.dma_start(t[:], t_dram[:])
                return t

            idx_t = [load(idxlo, [128, max(nch_cls[0] * 8, 8)], i16),
                     load(idxhi, [128, max(nch_cls[1] * 8, 8)], i16)]
            meta_t = load(meta, [128, 2 * ncht], f32)
            w_t = {n: load(w_in[n], [F, F], f16) for n in ("w0", "w1", "w2")}
            wm1_t = load(wm1, [F, NH], f16)
            wm2_t = [None] * 4
            for h in range(4):
                wm2_t[h] = res.tile([128, NO], f16, tag=f"wm2_{h}", name=f"wm2t{h}")
                nc.sync.dma_start(wm2_t[h][:], wm2[128 * h:128 * (h + 1), :])
            b_t = {n: load(b_in[n], [128, F], f32) for n in ("b0r", "b1r", "b2r")}
            bm1c_t = load(bm1c, [128, 4], f32)
            bm2r_t = load(bm2r, [128, NO], f32)
            iota_t = load(iota, [128, G], f16)
            bcol_t = load(bcol, [128, TILES], f32)
            invc_t = load(invc, [128, TILES], f32)

            layer_w = [("w0", "b0r", True), ("w1", "b1r", True), ("w2", "b2r", False)]

            for l in range(3):
                tbl = xtab if l == 0 else tabn
                tbl_ap = [tbl[0:LO, :], tbl[LO:NP, :] if HI > 0 else None]
                wname, bname, relu = layer_w[l]
                issued = [-1, -1]        # last issued batch per class
                cur = [None, None]       # current msg tile per class
                nbat = [-(-nch_cls[0] // BATCH_CH), -(-nch_cls[1] // BATCH_CH)]
                g = 0
                for w in range(NWIN):
                    width = min(WINW, SH - w * WINW)
                    chunks = schedule[w]
                    ps = wps.tile([128, WINW], f32, tag="wps")
                    for j, (cls, cid) in enumerate(chunks):
                        b, slab = divmod(cid, BATCH_CH)
                        if b != issued[cls]:
                            nb = min(BATCH_CH, nch_cls[cls] - b * BATCH_CH)
                            mt = msgp.tile([128, BATCH_CH, F], f16, tag=f"msg{cls}")
                            if variant == "memset":
                                nc.vector.memset(mt[:, :nb, :], 0.0)
                            elif variant not in ("nogather", "nogather_nocc"):
                                nc.gpsimd.dma_gather(
                                    mt[:, :nb, :], tbl_ap[cls],
                                    idx_t[cls][:, b * (BATCH_CH * 8):
                                               b * (BATCH_CH * 8) + nb * 8],
                                    nb * CH, nb * CH, F, single_packet=False)
                            issued[cls] = b
                            cur[cls] = mt
                        if variant in ("gatheronly", "gs", "gsm"):
                            g += 1
                            continue
                        S = sp.tile([128, WINW], f16, tag="S")
                        if variant == "gs":
                            nc.vector.tensor_scalar(
                                out=S[:, :width], in0=iota_t[:, :width],
                                scalar1=meta_t[:, 2 * g:2 * g + 1],
                                scalar2=meta_t[:, 2 * g + 1:2 * g + 2],
                                op0=mybir.AluOpType.is_equal,
                                op1=mybir.AluOpType.mult)
                            g += 1
                            continue
                        nc.vector.tensor_scalar(
                            out=S[:, :width], in0=iota_t[:, :width],
                            scalar1=meta_t[:, 2 * g:2 * g + 1],
                            scalar2=meta_t[:, 2 * g + 1:2 * g + 2],
                            op0=mybir.AluOpType.is_equal,
                            op1=mybir.AluOpType.mult)
                        nc.tensor.matmul(
                            out=ps[:, :width], lhsT=cur[cls][:, slab, :],
                            rhs=S[:, :width],
                            start=(j == 0), stop=(j == len(chunks) - 1))
                        g += 1
                    if variant in ("gatheronly", "gs"):
                        continue
                    aggT = aggp.tile([128, WINW], f16, tag="aggT")
                    if variant == "gsm":
                        nc.vector.tensor_copy(aggT[:, :width], ps[:, :width])
                        continue
                    nc.vector.tensor_copy(aggT[:, :width], ps[:, :width])
                    for sub in range(width // 128):
                        t_idx = w * (WINW // 128) + sub
                        hp = hps.tile([128, F], f32, tag="hp")
                        nc.tensor.matmul(
                            out=hp[:], lhsT=aggT[:, sub * 128:(sub + 1) * 128],
                            rhs=w_t[wname][:], start=True, stop=True)
                        tmp = tmpp.tile([128, F], f32, tag="htmp")
                        nc.vector.tensor_tensor(
                            out=tmp[:], in0=hp[:], in1=b_t[bname][:],
                            op=mybir.AluOpType.add)
                        dst_sl = shb[:, t_idx * F:(t_idx + 1) * F]
                        if relu:
                            nc.vector.tensor_scalar(
                                out=dst_sl, in0=tmp[:], scalar1=0.0, scalar2=None,
                                op0=mybir.AluOpType.max)
                        else:
                            nc.vector.tensor_copy(dst_sl, tmp[:])
                assert g == ncht
                if variant in ("gatheronly", "gs", "gsm"):
                    continue
                if l < 2:
                    nc.sync.dma_start(
                        shard_d.ap().rearrange("(t p) f -> p t f", p=128),
                        shb[:, :].rearrange("p (t f) -> p t f", f=F))
                    if variant not in ("nocc", "nogather_nocc"):
                        nc.gpsimd.collective_compute(
                            "AllGather", mybir.AluOpType.bypass,
                            replica_groups=[list(range(NCORES))],
                            ins=[shard_d[:].opt()], outs=[tabn[:].opt()])

            # ---- mean pool ----
            if variant in ("gatheronly", "gs", "gsm"):
                # touch shb so it exists; write zeros tile to out to keep outputs
                z = tmpp.tile([128, NO], f32, tag="ot", name="zot")
                nc.vector.memset(z[:], 0.0)
                nc.vector.tensor_copy(shb[:, 0:NO], z[:])
                for gh in range(G // 128):
                    nc.sync.dma_start(out[128 * gh:128 * (gh + 1), :], z[:])
                raise _SkipRest
            gp = gps.tile([128, G], f32, tag="gp")
            for t in range(TILES):
                Gt = sp.tile([128, G], f16, tag="S")
                nc.vector.tensor_scalar(
                    out=Gt[:], in0=iota_t[:],
                    scalar1=bcol_t[:, t:t + 1], scalar2=invc_t[:, t:t + 1],
                    op0=mybir.AluOpType.is_equal, op1=mybir.AluOpType.mult)
                nc.tensor.matmul(out=gp[:], lhsT=shb[:, t * F:(t + 1) * F],
                                 rhs=Gt[:], start=(t == 0), stop=(t == TILES - 1))
            gtile = tmpp.tile([128, G], f32, tag="gtile")
            nc.vector.tensor_copy(gtile[:], gp[:])
            nc.sync.dma_start(gt_in[:], gtile[:])
            if variant not in ("nocc", "nogather_nocc"):
                nc.gpsimd.collective_compute(
                    "AllReduce", mybir.AluOpType.add,
                    replica_groups=[list(range(NCORES))],
                    ins=[gt_in[:].opt()], outs=[gt_out[:].opt()])
            gt16 = tmpp.tile([128, G], f16, tag="gt16")
            gfull = tmpp.tile([128, G], f32, tag="gfull")
            nc.sync.dma_start(gfull[:], gt_out[:])
            nc.vector.tensor_copy(gt16[:], gfull[:])

            # ---- MLP ----
            mt16 = []
            for h in range(4):
                mp = mps.tile([128, G], f32, tag="mp")
                nc.tensor.matmul(out=mp[:], lhsT=wm1_t[:, 128 * h:128 * (h + 1)],
                                 rhs=gt16[:], start=True, stop=True)
                mtile = tmpp.tile([128, G], f16, tag=f"mt{h}", name=f"mtile{h}")
                nc.vector.tensor_scalar(
                    out=mtile[:], in0=mp[:], scalar1=bm1c_t[:, h:h + 1],
                    scalar2=0.0, op0=mybir.AluOpType.add, op1=mybir.AluOpType.max)
                mt16.append(mtile)
            for gh in range(G // 128):
                op = mps.tile([128, NO], f32, tag="mp", name="op")
                for h in range(4):
                    nc.tensor.matmul(
                        out=op[:], lhsT=mt16[h][:, 128 * gh:128 * (gh + 1)],
                        rhs=wm2_t[h][:], start=(h == 0), stop=(h == 3))
                ot = tmpp.tile([128, NO], f32, tag="ot")
                nc.vector.tensor_tensor(out=ot[:], in0=op[:], in1=bm2r_t[:],
                                        op=mybir.AluOpType.add)
                nc.sync.dma_start(out[128 * gh:128 * (gh + 1), :], ot[:])

    nc.compile()
    return nc


def _get_built(inputs):
    import hashlib
    h = hashlib.sha1()
    h.update(np.ascontiguousarray(inputs["edge_index"]).tobytes())
    h.update(np.ascontiguousarray(inputs["batch"]).tobytes())
    key = (tuple(sorted((k, v.shape, str(v.dtype)) for k, v in inputs.items())),
           h.hexdigest())
    if key not in _cache:
        geom, in_maps = _host_prep(**inputs)
        nc = _build_bass(geom)
        _cache[key] = (geom, nc)
    else:
        geom, nc = _cache[key]
        _, in_maps = _host_prep(**inputs)
    return geom, nc, in_maps


def kernel(**inputs):
    inputs = {k: np.asarray(v) for k, v in inputs.items()}
    geom, nc, in_maps = _get_built(inputs)
    from concourse.bass_utils import run_bass_kernel_spmd
    res = run_bass_kernel_spmd(nc, in_maps, list(range(NCORES)))
    return np.asarray(res.results[0]["out"])



# revision 5
# speedup vs baseline: 1.9174x; 1.9174x over previous
"""GCN encoder (3x GCNConv + mean-pool + MLP) as an 8-core Trainium2 Bass kernel.

v2: minimizes per-exec input bytes and device time.

Sharding: nodes/edges partitioned by destination-node owner (8 shards).
Tables are W-premultiplied: tab0 = X@W0 (computed on device from per-core
transposed x shards, AllGathered), tab_{l+1} = relu(agg_l + b_l) @ W_{l+1}.
Per layer: per-edge source rows are gathered from the table (fp16 DRAM) with
dma_gather, scatter-added into per-destination sums via PE matmul against a
one-hot selection matrix built on DVE from compact fp16 metadata. The psum
drain fuses bias+relu on the ACT engine in feature-major layout, and the
next-table matmul transposes to node-major for free. Final layer transposes
via PE for the mean-pool one-hot matmul; pooled sums are AllReduced; the MLP
is sharded over the hidden dim with a ReduceScatter of output partials, and
each core returns only its 32-graph slice of the output.
"""

import numpy as np

NCORES = 8
F = 128            # hidden width
G = 256            # number of graphs
NH = 512           # MLP hidden
NO = 256           # MLP out
CH = 128           # edges per chunk
BATCH_CH = 32      # chunks per dma_gather batch
WINW = 256         # dst nodes per PSUM accumulation window
XT_FP8 = True      # ship x shards as fp8e4m3 (halves xt upload)
SELF_LOCAL = True  # self-loop contributions from local SBUF tiles, not gather

_cache = {}


def _host_prep(x, edge_index, batch, W0, b0, W1, b1, W2, b2, Wm1, bm1, Wm2, bm2):
    N = x.shape[0]
    FI = x.shape[1]
    SH = -(-N // (NCORES * 128)) * 128      # shard size (nodes), 128-multiple
    NP = SH * NCORES
    TILES = SH // 128
    NWIN = -(-SH // WINW)
    LO = min(32768, NP)
    HI = NP - LO
    NHS = NH // NCORES                      # MLP hidden slice per core
    GS = G // NCORES                        # output graphs per core

    if SELF_LOCAL:
        src = np.asarray(edge_index[0], dtype=np.int64)
        dst = np.asarray(edge_index[1], dtype=np.int64)
        deg = (np.bincount(np.concatenate([dst, np.arange(N, dtype=np.int64)]),
                           minlength=N).astype(np.float32))
    else:
        src = np.concatenate([edge_index[0], np.arange(N, dtype=np.int64)])
        dst = np.concatenate([edge_index[1], np.arange(N, dtype=np.int64)])
        deg = np.bincount(dst, minlength=N).astype(np.float32)
    dis = np.where(deg > 0, 1.0 / np.sqrt(np.maximum(deg, 1.0)), 0.0).astype(np.float32)
    norm = dis[src] * dis[dst]

    # per-core edge selection, ordered by (window, class, dst)
    per_core = []
    for c in range(NCORES):
        base = c * SH
        sel = (dst >= base) & (dst < base + SH)
        es = src[sel].astype(np.int64)
        ed = (dst[sel] - base).astype(np.int64)
        en = norm[sel]
        cl = (es >= LO).astype(np.int64)
        wi = ed // WINW
        order = np.lexsort((ed, cl, wi))
        per_core.append((es[order], ed[order], en[order], cl[order], wi[order]))

    # chunk counts per (window, class), equalized across cores
    counts = np.zeros((NCORES, NWIN, 2), dtype=np.int64)
    for c in range(NCORES):
        _, _, _, cl, wi = per_core[c]
        for cls in (0, 1):
            counts[c, :, cls] = np.bincount(wi[cl == cls], minlength=NWIN)
    nch = -(-counts.max(axis=0) // CH)  # [NWIN, 2] chunks
    nch_cls = nch.sum(axis=0)          # total chunks per class
    ncht = int(nch.sum())

    # shared program schedule: windows -> list of (cls, cid)
    schedule = []
    cid_ctr = [0, 0]
    for w in range(NWIN):
        lst = []
        for cls in (0, 1):
            for _ in range(int(nch[w, cls])):
                lst.append((cls, cid_ctr[cls]))
                cid_ctr[cls] += 1
        schedule.append(lst)

    # per-core streams: compact idx [16, nch_cls*8] int16;
    # per-chunk metadata split: dst-local offsets (uint8) + edge norms (fp16)
    idx_streams = [[], []]
    dlqs, nrhs = [], []
    for c in range(NCORES):
        es, ed, en, cl, wi = per_core[c]
        idx_parts = [[], []]
        dlq = np.zeros((128, ncht), dtype=np.uint8)
        nrh = np.zeros((128, ncht), dtype=np.float16)
        g = 0
        pos = 0
        for w in range(NWIN):
            for cls in (0, 1):
                n_e = int(counts[c, w, cls])
                tot = int(nch[w, cls]) * CH
                ge, gd, gn = es[pos:pos + n_e], ed[pos:pos + n_e], en[pos:pos + n_e]
                pos += n_e
                pad = tot - n_e
                iv = ge - (LO if cls else 0)
                iv = np.concatenate([iv, np.zeros(pad, np.int64)])
                dl = np.concatenate([gd - w * WINW, np.zeros(pad, np.int64)])
                nr = np.concatenate([gn, np.zeros(pad, np.float32)])
                idx_parts[cls].append(iv.astype(np.int16))
                for k in range(tot // CH):
                    dlq[:, g] = dl[k * CH:(k + 1) * CH].astype(np.uint8)
                    nrh[:, g] = nr[k * CH:(k + 1) * CH].astype(np.float16)
                    g += 1
        assert g == ncht
        for cls in (0, 1):
            arr = (np.concatenate(idx_parts[cls]) if idx_parts[cls]
                   else np.zeros(0, np.int16))
            assert arr.size == nch_cls[cls] * CH
            if arr.size:
                wrapped = arr.reshape(-1, 16).T       # [16, nch_cls*8]
            else:
                wrapped = np.zeros((16, 8), np.int16)  # dummy
            idx_streams[cls].append(np.ascontiguousarray(wrapped))
        dlqs.append(dlq)
        nrhs.append(nrh)

    # pooling helpers
    cnt = np.bincount(batch.astype(np.int64), minlength=G).astype(np.float32)
    invc_all = (1.0 / np.maximum(cnt, 1.0))[batch.astype(np.int64)]
    selfnr_all = dis * dis
    bcols, invcs, selfnrs = [], [], []
    for c in range(NCORES):
        sl = slice(c * SH, min((c + 1) * SH, N))
        b_sh = np.zeros(SH, np.float32)
        i_sh = np.zeros(SH, np.float32)
        s_sh = np.zeros(SH, np.float32)
        nreal = max(0, min((c + 1) * SH, N) - c * SH)
        if nreal > 0:
            b_sh[:nreal] = batch[sl].astype(np.float32)
            i_sh[:nreal] = invc_all[sl].astype(np.float32)
            s_sh[:nreal] = selfnr_all[sl]
        bcols.append(np.ascontiguousarray(b_sh.reshape(TILES, 128).T))
        invcs.append(np.ascontiguousarray(i_sh.reshape(TILES, 128).T))
        selfnrs.append(np.ascontiguousarray(s_sh.reshape(TILES, 128).T))

    consts = {
        "w0": W0.astype(np.float16),                     # [FI, F]
        "w1": W1.astype(np.float16), "w2": W2.astype(np.float16),
        "bcols3": np.stack([b0, b1, b2], axis=1).astype(np.float32),  # [F, 3]
    }
    if XT_FP8:
        import ml_dtypes
        xt_np = ml_dtypes.float8_e4m3
    else:
        xt_np = np.float16
    in_maps = []
    for c in range(NCORES):
        m = dict(consts)
        lo = c * SH
        hi = min((c + 1) * SH, N)
        xt = np.zeros((FI, SH), dtype=xt_np)
        xt[:, :hi - lo] = x[lo:hi].T.astype(xt_np)
        m["xt"] = np.ascontiguousarray(xt)
        m["idxlo"] = idx_streams[0][c]
        m["idxhi"] = idx_streams[1][c]
        m["dlq"] = dlqs[c]
        m["nrh"] = nrhs[c]
        m["bcol"] = bcols[c]
        m["invc"] = invcs[c]
        m["selfnr"] = selfnrs[c]
        m["wm1s"] = np.ascontiguousarray(Wm1[:, c * NHS:(c + 1) * NHS]).astype(np.float16)
        m["wm2s"] = np.ascontiguousarray(Wm2[c * NHS:(c + 1) * NHS, :]).astype(np.float16)
        m["bm1s"] = np.ascontiguousarray(
            bm1[c * NHS:(c + 1) * NHS, None]).astype(np.float32)
        m["bm2s8"] = (bm2[None, :] / NCORES).astype(np.float16)
        in_maps.append(m)

    geom = dict(N=N, FI=FI, NP=NP, SH=SH, TILES=TILES, NWIN=NWIN, LO=LO, HI=HI,
                NHS=NHS, GS=GS, nch=nch, nch_cls=[int(v) for v in nch_cls],
                ncht=ncht, schedule=schedule)
    return geom, in_maps


class _SkipRest(Exception):
    pass


def _build_bass(geom, variant="full", gcfg=None):
    import concourse.bass as bass
    import concourse.tile as tile
    from concourse import bacc, mybir

    gcfg = dict(dict(batch=8, sp=False, nq=4, qg=True), **(gcfg or {}))
    BCH = gcfg["batch"]

    f16, f32, i16 = mybir.dt.float16, mybir.dt.float32, mybir.dt.int16
    u8 = mybir.dt.uint8
    fxt = mybir.dt.float8e4 if XT_FP8 else f16
    FI, NP, SH, TILES, NWIN = (geom["FI"], geom["NP"], geom["SH"],
                               geom["TILES"], geom["NWIN"])
    LO, HI, NHS, GS = geom["LO"], geom["HI"], geom["NHS"], geom["GS"]
    nch, nch_cls, ncht = geom["nch"], geom["nch_cls"], geom["ncht"]
    schedule = geom["schedule"]

    nc = bacc.Bacc("TRN2", target_bir_lowering=False, debug=False,
                   num_devices=NCORES, num_swdge_queues=gcfg["nq"])

    xt = nc.dram_tensor("xt", [FI, SH], fxt, kind="ExternalInput")
    idxlo = nc.dram_tensor("idxlo", [16, max(nch_cls[0] * 8, 8)], i16,
                           kind="ExternalInput")
    idxhi = nc.dram_tensor("idxhi", [16, max(nch_cls[1] * 8, 8)], i16,
                           kind="ExternalInput")
    dlq = nc.dram_tensor("dlq", [128, ncht], u8, kind="ExternalInput")
    nrh = nc.dram_tensor("nrh", [128, ncht], f16, kind="ExternalInput")
    w0 = nc.dram_tensor("w0", [FI, F], f16, kind="ExternalInput")
    w1 = nc.dram_tensor("w1", [F, F], f16, kind="ExternalInput")
    w2 = nc.dram_tensor("w2", [F, F], f16, kind="ExternalInput")
    bcols3 = nc.dram_tensor("bcols3", [F, 3], f32, kind="ExternalInput")
    wm1s = nc.dram_tensor("wm1s", [F, NHS], f16, kind="ExternalInput")
    wm2s = nc.dram_tensor("wm2s", [NHS, NO], f16, kind="ExternalInput")
    bm1s = nc.dram_tensor("bm1s", [NHS, 1], f32, kind="ExternalInput")
    bm2s8 = nc.dram_tensor("bm2s8", [1, NO], f16, kind="ExternalInput")
    bcol = nc.dram_tensor("bcol", [128, TILES], f32, kind="ExternalInput")
    invc = nc.dram_tensor("invc", [128, TILES], f32, kind="ExternalInput")
    selfnr = (nc.dram_tensor("selfnr", [128, TILES], f32, kind="ExternalInput")
              if SELF_LOCAL else None)
    out = nc.dram_tensor("out", [GS, NO], f32, kind="ExternalOutput")

    shard_d = nc.dram_tensor("shard_d", [SH, F], f16)
    tabs = [nc.dram_tensor(f"tab{l}", [NP, F], f16, addr_space="Shared")
            for l in range(3)]
    gt_in = nc.dram_tensor("gt_in", [128, G], f32)
    gt_out = nc.dram_tensor("gt_out", [128, G], f32, addr_space="Shared")
    part_d = nc.dram_tensor("part_d", [G, NO], f32)
    rs_out = nc.dram_tensor("rs_out", [GS, NO], f32)

    shb = nc.alloc_sbuf_tensor("shb", [128, TILES * F], f16)

    relu_fn = mybir.ActivationFunctionType.Relu

    import contextlib
    with tile.TileContext(nc) as tc:
        with (
            contextlib.suppress(_SkipRest),
            tc.tile_pool(name="res", bufs=1) as res,
            tc.tile_pool(name="msg", bufs=3) as msgp,
            tc.tile_pool(name="sp", bufs=4) as sp,
            tc.tile_pool(name="agg", bufs=2) as aggp,
            tc.tile_pool(name="tmp", bufs=2) as tmpp,
            tc.tile_pool(name="wps", bufs=2, space="PSUM") as wps,
            tc.tile_pool(name="hps", bufs=2, space="PSUM") as hps,
            tc.tile_pool(name="gps", bufs=1, space="PSUM") as gps,
            tc.tile_pool(name="mps", bufs=1, space="PSUM") as mps,
        ):
            # ---- resident loads ----
            def load(t_dram, shape, dtype):
                t = res.tile(shape, dtype, tag=t_dram.name)
                nc.sync.dma_start(t[:], t_dram[:])
                return t

            idx_t = []
            for cls, t_dram in ((0, idxlo), (1, idxhi)):
                w = max(nch_cls[cls] * 8, 8)
                t = res.tile([128, w], i16, tag=f"idx{cls}", name=f"idxt{cls}")
                for k in range(8):
                    nc.sync.dma_start(t[16 * k:16 * (k + 1), :], t_dram[:])
                idx_t.append(t)
            dlq_t = load(dlq, [128, ncht], u8)
            nrh_t = load(nrh, [128, ncht], f16)
            dl32 = res.tile([128, ncht], f32, tag="dl32")
            nc.vector.tensor_copy(dl32[:], dlq_t[:])
            nr32 = res.tile([128, ncht], f32, tag="nr32")
            nc.vector.tensor_copy(nr32[:], nrh_t[:])
            xt_t = load(xt, [FI, SH], fxt)
            w0_t = load(w0, [FI, F], f16)
            w_t = {1: load(w1, [F, F], f16), 2: load(w2, [F, F], f16)}
            bcols3_t = load(bcols3, [F, 3], f32)
            wm1s_t = load(wm1s, [F, NHS], f16)
            wm2s_t = load(wm2s, [NHS, NO], f16)
            bm1s_t = load(bm1s, [NHS, 1], f32)
            bm2s8_t = load(bm2s8, [1, NO], f16)
            bcol_t = load(bcol, [128, TILES], f32)
            invc_t = load(invc, [128, TILES], f32)
            selfnr_t = load(selfnr, [128, TILES], f32) if SELF_LOCAL else None

            # iota [128, G] fp16 (values 0..G-1 per row), built on device
            io16 = res.tile([128, G], i16, tag="io16")
            nc.gpsimd.iota(io16[:], pattern=[[1, G]], base=0,
                           channel_multiplier=0)
            iota_t = res.tile([128, G], f16, tag="iota")
            nc.vector.tensor_copy(iota_t[:], io16[:])
            # identity [128,128] fp16 for PE transpose
            icol16 = res.tile([128, 1], i16, tag="icol16")
            nc.gpsimd.iota(icol16[:], pattern=[[0, 1]], base=0,
                           channel_multiplier=1)
            icolf = res.tile([128, 1], f32, tag="icolf")
            nc.vector.tensor_copy(icolf[:], icol16[:])
            ident = res.tile([128, 128], f16, tag="ident")
            nc.vector.tensor_scalar(
                out=ident[:], in0=iota_t[:, 0:128],
                scalar1=icolf[:], scalar2=None,
                op0=mybir.AluOpType.is_equal)
            ones1 = res.tile([1, 128], f16, tag="ones1")
            nc.vector.memset(ones1[:], 1.0)

            # ---- T0 = X @ W0 (per-shard), node-major into shb ----
            for t in range(TILES):
                t0p = hps.tile([128, F], f32, tag="hp")
                nc.tensor.matmul(out=t0p[:], lhsT=xt_t[:, 128 * t:128 * (t + 1)],
                                 rhs=w0_t[:], start=True, stop=True)
                nc.scalar.copy(out=shb[:, t * F:(t + 1) * F], in_=t0p[:])
            nc.sync.dma_start(
                shard_d.ap().rearrange("(t p) f -> p t f", p=128),
                shb[:, :].rearrange("p (t f) -> p t f", f=F))
            if variant not in ("nocc",):
                nc.gpsimd.collective_compute(
                    "AllGather", mybir.AluOpType.bypass,
                    replica_groups=[list(range(NCORES))],
                    ins=[shard_d[:].opt()], outs=[tabs[0][:].opt()])

            # ---- 3 GCN layers ----
            gctr = [0]  # global gather counter for queue round-robin
            for l in range(3):
                tbl = tabs[l]
                tbl_ap = [tbl[0:LO, :], tbl[LO:NP, :] if HI > 0 else None]
                issued = [-1, -1]
                cur = [None, None]
                g = 0
                for w in range(NWIN):
                    width = min(WINW, SH - w * WINW)
                    chunks = schedule[w]
                    ops = [("c", x) for x in chunks]
                    use_self = SELF_LOCAL and variant not in ("gatheronly", "gs")
                    if use_self:
                        selfops = [("s", sub) for sub in range(width // 128)]
                        ops = (ops[:1] + selfops + ops[1:]) if ops else selfops
                    ps = wps.tile([128, WINW], f32, tag="wps")
                    no_chunks = not chunks
                    for j, op in enumerate(ops):
                        first, last = (j == 0), (j == len(ops) - 1)
                        if op[0] == "s":
                            sub = op[1]
                            t_idx = w * (WINW // 128) + sub
                            Sd = sp.tile([128, WINW], f16, tag="S")
                            nc.vector.tensor_scalar(
                                out=Sd[:, :128], in0=iota_t[:, :128],
                                scalar1=icolf[:],
                                scalar2=selfnr_t[:, t_idx:t_idx + 1],
                                op0=mybir.AluOpType.is_equal,
                                op1=mybir.AluOpType.mult)
                            nc.tensor.matmul(
                                out=ps[:, sub * 128:(sub + 1) * 128],
                                lhsT=shb[:, t_idx * F:(t_idx + 1) * F],
                                rhs=Sd[:, :128],
                                start=(True if no_chunks else False),
                                stop=last, skip_group_check=True)
                            continue
                        cls, cid = op[1]
                        b, slab = divmod(cid, BCH)
                        if b != issued[cls]:
                            nb = min(BCH, nch_cls[cls] - b * BCH)
                            mt = msgp.tile([128, BCH, F], f16, tag=f"msg{cls}")
                            if variant == "memset":
                                nc.vector.memset(mt[:, :nb, :], 0.0)
                            else:
                                qn = ((gctr[0] if gcfg["qg"] else b)
                                      % gcfg["nq"])
                                gctr[0] += 1
                                nc.gpsimd.dma_gather(
                                    mt[:, :nb, :], tbl_ap[cls],
                                    idx_t[cls][:, b * (BCH * 8):
                                               b * (BCH * 8) + nb * 8],
                                    nb * CH, nb * CH, F,
                                    single_packet=gcfg["sp"],
                                    queue_num=qn)
                            issued[cls] = b
                            cur[cls] = mt
                        if variant == "gatheronly":
                            g += 1
                            continue
                        S = sp.tile([128, WINW], f16, tag="S")
                        nc.vector.tensor_scalar(
                            out=S[:, :width], in0=iota_t[:, :width],
                            scalar1=dl32[:, g:g + 1],
                            scalar2=nr32[:, g:g + 1],
                            op0=mybir.AluOpType.is_equal,
                            op1=mybir.AluOpType.mult)
                        if variant == "gs":
                            g += 1
                            continue
                        nc.tensor.matmul(
                            out=ps[:, :width], lhsT=cur[cls][:, slab, :],
                            rhs=S[:, :width],
                            start=first, stop=last)
                        g += 1
                    if variant in ("gatheronly", "gs"):
                        continue
                    for sub in range(width // 128):
                        t_idx = w * (WINW // 128) + sub
                        pslice = ps[:, sub * 128:(sub + 1) * 128]
                        dst_sl = shb[:, t_idx * F:(t_idx + 1) * F]
                        if l < 2:
                            # relu(agg + b_l) in feature-major, then @W_{l+1}
                            # transposes to node-major for the next table
                            aggT = aggp.tile([128, 128], f16, tag="aggT")
                            nc.scalar.activation(
                                out=aggT[:], in_=pslice, func=relu_fn,
                                bias=bcols3_t[:, l:l + 1])
                            if variant == "gsm":
                                continue
                            tp = hps.tile([128, F], f32, tag="hp")
                            nc.tensor.matmul(out=tp[:], lhsT=aggT[:],
                                             rhs=w_t[l + 1][:],
                                             start=True, stop=True)
                            nc.scalar.copy(out=dst_sl, in_=tp[:])
                        else:
                            # H3^T = agg + b2 (no relu), transpose to
                            # node-major for pooling
                            aggT = aggp.tile([128, 128], f16, tag="aggT")
                            nc.scalar.add(out=aggT[:], in_=pslice,
                                          add=bcols3_t[:, 2:3])
                            if variant == "gsm":
                                continue
                            tp = hps.tile([128, 128], f16, tag="hp",
                                          name="tppose")
                            nc.tensor.transpose(out=tp[:], in_=aggT[:],
                                                identity=ident[:])
                            nc.scalar.copy(out=dst_sl, in_=tp[:])
                assert g == ncht
                if variant in ("gatheronly", "gs", "gsm"):
                    continue
                if l < 2:
                    nc.sync.dma_start(
                        shard_d.ap().rearrange("(t p) f -> p t f", p=128),
                        shb[:, :].rearrange("p (t f) -> p t f", f=F))
                    if variant not in ("nocc",):
                        nc.gpsimd.collective_compute(
                            "AllGather", mybir.AluOpType.bypass,
                            replica_groups=[list(range(NCORES))],
                            ins=[shard_d[:].opt()], outs=[tabs[l + 1][:].opt()])

            # ---- mean pool ----
            if variant in ("gatheronly", "gs", "gsm"):
                z = tmpp.tile([GS, NO], f32, tag="ot", name="zot")
                nc.vector.memset(z[:], 0.0)
                nc.sync.dma_start(out[:], z[:])
                raise _SkipRest
            gp = gps.tile([128, G], f32, tag="gp")
            for t in range(TILES):
                Gt = sp.tile([128, G], f16, tag="S")
                nc.vector.tensor_scalar(
                    out=Gt[:], in0=iota_t[:],
                    scalar1=bcol_t[:, t:t + 1], scalar2=invc_t[:, t:t + 1],
                    op0=mybir.AluOpType.is_equal, op1=mybir.AluOpType.mult)
                nc.tensor.matmul(out=gp[:], lhsT=shb[:, t * F:(t + 1) * F],
                                 rhs=Gt[:], start=(t == 0), stop=(t == TILES - 1))
            gtile = tmpp.tile([128, G], f32, tag="gtile")
            nc.scalar.copy(out=gtile[:], in_=gp[:])
            nc.sync.dma_start(gt_in[:], gtile[:])
            if variant not in ("nocc",):
                nc.gpsimd.collective_compute(
                    "AllReduce", mybir.AluOpType.add,
                    replica_groups=[list(range(NCORES))],
                    ins=[gt_in[:].opt()], outs=[gt_out[:].opt()])
            gfull = tmpp.tile([128, G], f32, tag="gfull")
            nc.sync.dma_start(gfull[:], gt_out[:])
            gt16 = tmpp.tile([128, G], f16, tag="gt16")
            nc.vector.tensor_copy(gt16[:], gfull[:])

            # ---- MLP (hidden dim sharded; partials reduce-scattered) ----
            mp = mps.tile([NHS, G], f32, tag="mp")
            nc.tensor.matmul(out=mp[:], lhsT=wm1s_t[:], rhs=gt16[:],
                             start=True, stop=True)
            mt16 = tmpp.tile([NHS, G], f16, tag="mt16")
            nc.scalar.activation(out=mt16[:], in_=mp[:], func=relu_fn,
                                 bias=bm1s_t[:])
            for gh in range(G // 128):
                op = mps.tile([128, NO], f32, tag="op")
                nc.tensor.matmul(
                    out=op[:], lhsT=mt16[:, 128 * gh:128 * (gh + 1)],
                    rhs=wm2s_t[:], start=True, stop=False)
                nc.tensor.matmul(
                    out=op[:], lhsT=ones1[:], rhs=bm2s8_t[:],
                    start=False, stop=True)
                ot = tmpp.tile([128, NO], f32, tag="ot")
                nc.scalar.copy(out=ot[:], in_=op[:])
                nc.sync.dma_start(part_d[128 * gh:128 * (gh + 1), :], ot[:])
            if variant not in ("nocc",):
                nc.gpsimd.collective_compute(
                    "ReduceScatter", mybir.AluOpType.add,
                    replica_groups=[list(range(NCORES))],
                    ins=[part_d[:].opt()], outs=[rs_out[:].opt()])
            rt = tmpp.tile([GS, NO], f32, tag="rt")
            nc.sync.dma_start(rt[:], rs_out[:])
            nc.sync.dma_start(out[:], rt[:])

    nc.compile()
    return nc


def _get_built(inputs, variant="full"):
    import hashlib
    h = hashlib.sha1()
    h.update(np.ascontiguousarray(inputs["edge_index"]).tobytes())
    h.update(np.ascontiguousarray(inputs["batch"]).tobytes())
    key = (variant,
           tuple(sorted((k, v.shape, str(v.dtype)) for k, v in inputs.items())),
           h.hexdigest())
    if key not in _cache:
        geom, in_maps = _host_prep(**inputs)
        nc = _build_bass(geom, variant)
        _cache[key] = (geom, nc)
    else:
        geom, nc = _cache[key]
        _, in_maps = _host_prep(**inputs)
    return geom, nc, in_maps


def kernel(**inputs):
    inputs = {k: np.asarray(v) for k, v in inputs.items()}
    geom, nc, in_maps = _get_built(inputs)
    from concourse.bass_utils import run_bass_kernel_spmd
    res = run_bass_kernel_spmd(nc, in_maps, list(range(NCORES)))
    return np.concatenate([np.asarray(res.results[c]["out"])
                           for c in range(NCORES)], axis=0)


# revision 6
# speedup vs baseline: 2.7414x; 1.4298x over previous
"""GCN encoder (3x GCNConv + mean-pool + MLP) as an 8-core Trainium2 Bass kernel.

v2: minimizes per-exec input bytes and device time.

Sharding: nodes/edges partitioned by destination-node owner (8 shards).
Tables are W-premultiplied: tab0 = X@W0 (computed on device from per-core
transposed x shards, AllGathered), tab_{l+1} = relu(agg_l + b_l) @ W_{l+1}.
Per layer: per-edge source rows are gathered from the table (fp16 DRAM) with
dma_gather, scatter-added into per-destination sums via PE matmul against a
one-hot selection matrix built on DVE from compact fp16 metadata. The psum
drain fuses bias+relu on the ACT engine in feature-major layout, and the
next-table matmul transposes to node-major for free. Final layer transposes
via PE for the mean-pool one-hot matmul; pooled sums are AllReduced; the MLP
is sharded over the hidden dim with a ReduceScatter of output partials, and
each core returns only its 32-graph slice of the output.
"""

import numpy as np

NCORES = 8
F = 128            # hidden width
G = 256            # number of graphs
NH = 512           # MLP hidden
NO = 256           # MLP out
CH = 128           # edges per chunk
BATCH_CH = 32      # chunks per dma_gather batch
WINW = 256         # dst nodes per PSUM accumulation window
XT_FP8 = True      # ship x shards as fp8e4m3 (halves xt upload)
SELF_LOCAL = True  # self-loop contributions from local SBUF tiles, not gather

_cache = {}


def _host_prep(x, edge_index, batch, W0, b0, W1, b1, W2, b2, Wm1, bm1, Wm2, bm2):
    N = x.shape[0]
    FI = x.shape[1]
    SH = -(-N // (NCORES * 128)) * 128      # shard size (nodes), 128-multiple
    NP = SH * NCORES
    TILES = SH // 128
    NWIN = -(-SH // WINW)
    LO = min(32768, NP)
    HI = NP - LO
    NHS = NH // NCORES                      # MLP hidden slice per core
    GS = G // NCORES                        # output graphs per core

    if SELF_LOCAL:
        src = np.asarray(edge_index[0], dtype=np.int64)
        dst = np.asarray(edge_index[1], dtype=np.int64)
        deg = (np.bincount(np.concatenate([dst, np.arange(N, dtype=np.int64)]),
                           minlength=N).astype(np.float32))
    else:
        src = np.concatenate([edge_index[0], np.arange(N, dtype=np.int64)])
        dst = np.concatenate([edge_index[1], np.arange(N, dtype=np.int64)])
        deg = np.bincount(dst, minlength=N).astype(np.float32)
    dis = np.where(deg > 0, 1.0 / np.sqrt(np.maximum(deg, 1.0)), 0.0).astype(np.float32)
    norm = dis[src] * dis[dst]

    # per-core edge selection, ordered by (window, class, dst)
    per_core = []
    for c in range(NCORES):
        base = c * SH
        sel = (dst >= base) & (dst < base + SH)
        es = src[sel].astype(np.int64)
        ed = (dst[sel] - base).astype(np.int64)
        en = norm[sel]
        cl = (es >= LO).astype(np.int64)
        wi = ed // WINW
        order = np.lexsort((ed, cl, wi))
        per_core.append((es[order], ed[order], en[order], cl[order], wi[order]))

    # chunk counts per (window, class), equalized across cores
    counts = np.zeros((NCORES, NWIN, 2), dtype=np.int64)
    for c in range(NCORES):
        _, _, _, cl, wi = per_core[c]
        for cls in (0, 1):
            counts[c, :, cls] = np.bincount(wi[cl == cls], minlength=NWIN)
    nch = -(-counts.max(axis=0) // CH)  # [NWIN, 2] chunks
    nch_cls = nch.sum(axis=0)          # total chunks per class
    ncht = int(nch.sum())

    # shared program schedule: windows -> list of (cls, cid)
    schedule = []
    cid_ctr = [0, 0]
    for w in range(NWIN):
        lst = []
        for cls in (0, 1):
            for _ in range(int(nch[w, cls])):
                lst.append((cls, cid_ctr[cls]))
                cid_ctr[cls] += 1
        schedule.append(lst)

    # per-core streams: compact idx [16, nch_cls*8] int16;
    # per-chunk metadata split: dst-local offsets (uint8) + edge norms (fp16)
    idx_streams = [[], []]
    dlqs, nrhs = [], []
    for c in range(NCORES):
        es, ed, en, cl, wi = per_core[c]
        idx_parts = [[], []]
        dlq = np.zeros((128, ncht), dtype=np.uint8)
        nrh = np.zeros((128, ncht), dtype=np.float16)
        g = 0
        pos = 0
        for w in range(NWIN):
            for cls in (0, 1):
                n_e = int(counts[c, w, cls])
                tot = int(nch[w, cls]) * CH
                ge, gd, gn = es[pos:pos + n_e], ed[pos:pos + n_e], en[pos:pos + n_e]
                pos += n_e
                pad = tot - n_e
                iv = ge - (LO if cls else 0)
                iv = np.concatenate([iv, np.zeros(pad, np.int64)])
                dl = np.concatenate([gd - w * WINW, np.zeros(pad, np.int64)])
                nr = np.concatenate([gn, np.zeros(pad, np.float32)])
                idx_parts[cls].append(iv.astype(np.int16))
                for k in range(tot // CH):
                    dlq[:, g] = dl[k * CH:(k + 1) * CH].astype(np.uint8)
                    nrh[:, g] = nr[k * CH:(k + 1) * CH].astype(np.float16)
                    g += 1
        assert g == ncht
        for cls in (0, 1):
            arr = (np.concatenate(idx_parts[cls]) if idx_parts[cls]
                   else np.zeros(0, np.int16))
            assert arr.size == nch_cls[cls] * CH
            if arr.size:
                wrapped = arr.reshape(-1, 16).T       # [16, nch_cls*8]
            else:
                wrapped = np.zeros((16, 8), np.int16)  # dummy
            idx_streams[cls].append(np.ascontiguousarray(wrapped))
        dlqs.append(dlq)
        nrhs.append(nrh)

    # pooling helpers
    cnt = np.bincount(batch.astype(np.int64), minlength=G).astype(np.float32)
    invc_all = (1.0 / np.maximum(cnt, 1.0))[batch.astype(np.int64)]
    selfnr_all = dis * dis
    bcols, invcs, selfnrs = [], [], []
    for c in range(NCORES):
        sl = slice(c * SH, min((c + 1) * SH, N))
        b_sh = np.zeros(SH, np.float32)
        i_sh = np.zeros(SH, np.float32)
        s_sh = np.zeros(SH, np.float32)
        nreal = max(0, min((c + 1) * SH, N) - c * SH)
        if nreal > 0:
            b_sh[:nreal] = batch[sl].astype(np.float32)
            i_sh[:nreal] = invc_all[sl].astype(np.float32)
            s_sh[:nreal] = selfnr_all[sl]
        bcols.append(np.ascontiguousarray(b_sh.reshape(TILES, 128).T))
        invcs.append(np.ascontiguousarray(i_sh.reshape(TILES, 128).T))
        selfnrs.append(np.ascontiguousarray(s_sh.reshape(TILES, 128).T))

    consts = {
        "w0": W0.astype(np.float16),                     # [FI, F]
        "w1": W1.astype(np.float16), "w2": W2.astype(np.float16),
        "bcols3": np.stack([b0, b1, b2], axis=1).astype(np.float32),  # [F, 3]
    }
    if XT_FP8:
        import ml_dtypes
        xt_np = ml_dtypes.float8_e4m3
    else:
        xt_np = np.float16
    in_maps = []
    for c in range(NCORES):
        m = dict(consts)
        lo = c * SH
        hi = min((c + 1) * SH, N)
        xt = np.zeros((FI, SH), dtype=xt_np)
        xt[:, :hi - lo] = x[lo:hi].T.astype(xt_np)
        m["xt"] = np.ascontiguousarray(xt)
        m["idxlo"] = idx_streams[0][c]
        m["idxhi"] = idx_streams[1][c]
        m["dlq"] = dlqs[c]
        m["nrh"] = nrhs[c]
        m["bcol"] = bcols[c]
        m["invc"] = invcs[c]
        m["selfnr"] = selfnrs[c]
        m["wm1s"] = np.ascontiguousarray(Wm1[:, c * NHS:(c + 1) * NHS]).astype(np.float16)
        m["wm2s"] = np.ascontiguousarray(Wm2[c * NHS:(c + 1) * NHS, :]).astype(np.float16)
        m["bm1s"] = np.ascontiguousarray(
            bm1[c * NHS:(c + 1) * NHS, None]).astype(np.float32)
        m["bm2s8"] = (bm2[None, :] / NCORES).astype(np.float16)
        in_maps.append(m)

    geom = dict(N=N, FI=FI, NP=NP, SH=SH, TILES=TILES, NWIN=NWIN, LO=LO, HI=HI,
                NHS=NHS, GS=GS, nch=nch, nch_cls=[int(v) for v in nch_cls],
                ncht=ncht, schedule=schedule)
    return geom, in_maps


class _SkipRest(Exception):
    pass


def _build_bass(geom, variant="full", gcfg=None):
    import concourse.bass as bass
    import concourse.tile as tile
    from concourse import bacc, mybir

    gcfg = dict(dict(batch=8, sp=False, nq=4, qg=True), **(gcfg or {}))
    BCH = gcfg["batch"]

    f16, f32, i16 = mybir.dt.float16, mybir.dt.float32, mybir.dt.int16
    u8 = mybir.dt.uint8
    fxt = mybir.dt.float8e4 if XT_FP8 else f16
    FI, NP, SH, TILES, NWIN = (geom["FI"], geom["NP"], geom["SH"],
                               geom["TILES"], geom["NWIN"])
    LO, HI, NHS, GS = geom["LO"], geom["HI"], geom["NHS"], geom["GS"]
    nch, nch_cls, ncht = geom["nch"], geom["nch_cls"], geom["ncht"]
    schedule = geom["schedule"]

    nc = bacc.Bacc("TRN2", target_bir_lowering=False, debug=False,
                   num_devices=NCORES, num_swdge_queues=gcfg["nq"])

    xt = nc.dram_tensor("xt", [FI, SH], fxt, kind="ExternalInput")
    idxlo = nc.dram_tensor("idxlo", [16, max(nch_cls[0] * 8, 8)], i16,
                           kind="ExternalInput")
    idxhi = nc.dram_tensor("idxhi", [16, max(nch_cls[1] * 8, 8)], i16,
                           kind="ExternalInput")
    dlq = nc.dram_tensor("dlq", [128, ncht], u8, kind="ExternalInput")
    nrh = nc.dram_tensor("nrh", [128, ncht], f16, kind="ExternalInput")
    w0 = nc.dram_tensor("w0", [FI, F], f16, kind="ExternalInput")
    w1 = nc.dram_tensor("w1", [F, F], f16, kind="ExternalInput")
    w2 = nc.dram_tensor("w2", [F, F], f16, kind="ExternalInput")
    bcols3 = nc.dram_tensor("bcols3", [F, 3], f32, kind="ExternalInput")
    wm1s = nc.dram_tensor("wm1s", [F, NHS], f16, kind="ExternalInput")
    wm2s = nc.dram_tensor("wm2s", [NHS, NO], f16, kind="ExternalInput")
    bm1s = nc.dram_tensor("bm1s", [NHS, 1], f32, kind="ExternalInput")
    bm2s8 = nc.dram_tensor("bm2s8", [1, NO], f16, kind="ExternalInput")
    bcol = nc.dram_tensor("bcol", [128, TILES], f32, kind="ExternalInput")
    invc = nc.dram_tensor("invc", [128, TILES], f32, kind="ExternalInput")
    selfnr = (nc.dram_tensor("selfnr", [128, TILES], f32, kind="ExternalInput")
              if SELF_LOCAL else None)
    out = nc.dram_tensor("out", [GS, NO], f32, kind="ExternalOutput")

    shard_d = nc.dram_tensor("shard_d", [SH, F], f16)
    tabs = [nc.dram_tensor(f"tab{l}", [NP, F], f16, addr_space="Shared")
            for l in range(3)]
    gt_in = nc.dram_tensor("gt_in", [128, G], f32)
    gt_out = nc.dram_tensor("gt_out", [128, G], f32, addr_space="Shared")
    part_d = nc.dram_tensor("part_d", [G, NO], f32)
    rs_out = nc.dram_tensor("rs_out", [GS, NO], f32)

    shb = nc.alloc_sbuf_tensor("shb", [128, TILES * F], f16)

    relu_fn = mybir.ActivationFunctionType.Relu

    import contextlib
    with tile.TileContext(nc) as tc:
        with (
            contextlib.suppress(_SkipRest),
            tc.tile_pool(name="res", bufs=1) as res,
            tc.tile_pool(name="msg", bufs=6) as msgp,
            tc.tile_pool(name="sp", bufs=6) as sp,
            tc.tile_pool(name="agg", bufs=2) as aggp,
            tc.tile_pool(name="tmp", bufs=2) as tmpp,
            tc.tile_pool(name="wps", bufs=3, space="PSUM") as wps,
            tc.tile_pool(name="hps", bufs=2, space="PSUM") as hps,
            tc.tile_pool(name="gps", bufs=1, space="PSUM") as gps,
            tc.tile_pool(name="mps", bufs=1, space="PSUM") as mps,
        ):
            # ---- resident loads ----
            def load(t_dram, shape, dtype):
                t = res.tile(shape, dtype, tag=t_dram.name)
                nc.sync.dma_start(t[:], t_dram[:])
                return t

            idx_t = []
            for cls, t_dram in ((0, idxlo), (1, idxhi)):
                w = max(nch_cls[cls] * 8, 8)
                t = res.tile([128, w], i16, tag=f"idx{cls}", name=f"idxt{cls}")
                for k in range(8):
                    nc.sync.dma_start(t[16 * k:16 * (k + 1), :], t_dram[:])
                idx_t.append(t)
            dlq_t = load(dlq, [128, ncht], u8)
            nrh_t = load(nrh, [128, ncht], f16)
            dl32 = res.tile([128, ncht], f32, tag="dl32")
            nc.vector.tensor_copy(dl32[:], dlq_t[:])
            nr32 = res.tile([128, ncht], f32, tag="nr32")
            nc.vector.tensor_copy(nr32[:], nrh_t[:])
            xt_t = load(xt, [FI, SH], fxt)
            w0_t = load(w0, [FI, F], f16)
            w_t = {1: load(w1, [F, F], f16), 2: load(w2, [F, F], f16)}
            bcols3_t = load(bcols3, [F, 3], f32)
            wm1s_t = load(wm1s, [F, NHS], f16)
            wm2s_t = load(wm2s, [NHS, NO], f16)
            bm1s_t = load(bm1s, [NHS, 1], f32)
            bm2s8_t = load(bm2s8, [1, NO], f16)
            bcol_t = load(bcol, [128, TILES], f32)
            invc_t = load(invc, [128, TILES], f32)
            selfnr_t = load(selfnr, [128, TILES], f32) if SELF_LOCAL else None

            # iota [128, G] fp16 (values 0..G-1 per row), built on device
            io16 = res.tile([128, G], i16, tag="io16")
            nc.gpsimd.iota(io16[:], pattern=[[1, G]], base=0,
                           channel_multiplier=0)
            iota_t = res.tile([128, G], f16, tag="iota")
            nc.vector.tensor_copy(iota_t[:], io16[:])
            # identity [128,128] fp16 for PE transpose
            icol16 = res.tile([128, 1], i16, tag="icol16")
            nc.gpsimd.iota(icol16[:], pattern=[[0, 1]], base=0,
                           channel_multiplier=1)
            icolf = res.tile([128, 1], f32, tag="icolf")
            nc.vector.tensor_copy(icolf[:], icol16[:])
            ident = res.tile([128, 128], f16, tag="ident")
            nc.vector.tensor_scalar(
                out=ident[:], in0=iota_t[:, 0:128],
                scalar1=icolf[:], scalar2=None,
                op0=mybir.AluOpType.is_equal)
            ones1 = res.tile([1, 128], f16, tag="ones1")
            nc.vector.memset(ones1[:], 1.0)

            # ---- T0 = X @ W0 (per-shard), node-major into shb ----
            for t in range(TILES):
                t0p = hps.tile([128, F], f32, tag="hp")
                nc.tensor.matmul(out=t0p[:], lhsT=xt_t[:, 128 * t:128 * (t + 1)],
                                 rhs=w0_t[:], start=True, stop=True)
                nc.scalar.copy(out=shb[:, t * F:(t + 1) * F], in_=t0p[:])
            nc.sync.dma_start(
                shard_d.ap().rearrange("(t p) f -> p t f", p=128),
                shb[:, :].rearrange("p (t f) -> p t f", f=F))
            if variant not in ("nocc",):
                nc.gpsimd.collective_compute(
                    "AllGather", mybir.AluOpType.bypass,
                    replica_groups=[list(range(NCORES))],
                    ins=[shard_d[:].opt()], outs=[tabs[0][:].opt()])

            # ---- 3 GCN layers ----
            gctr = [0]  # global gather counter for queue round-robin
            for l in range(3):
                tbl = tabs[l]
                tbl_ap = [tbl[0:LO, :], tbl[LO:NP, :] if HI > 0 else None]
                issued = [-1, -1]
                cur = [None, None]
                g = 0
                for w in range(NWIN):
                    width = min(WINW, SH - w * WINW)
                    chunks = schedule[w]
                    ops = [("c", x) for x in chunks]
                    use_self = SELF_LOCAL and variant not in ("gatheronly", "gs")
                    if use_self:
                        selfops = [("s", sub) for sub in range(width // 128)]
                        ops = (ops[:1] + selfops + ops[1:]) if ops else selfops
                    ps = wps.tile([128, WINW], f32, tag="wps")
                    no_chunks = not chunks
                    for j, op in enumerate(ops):
                        first, last = (j == 0), (j == len(ops) - 1)
                        if op[0] == "s":
                            sub = op[1]
                            t_idx = w * (WINW // 128) + sub
                            Sd = sp.tile([128, WINW], f16, tag="S")
                            nc.vector.tensor_scalar(
                                out=Sd[:, :128], in0=iota_t[:, :128],
                                scalar1=icolf[:],
                                scalar2=selfnr_t[:, t_idx:t_idx + 1],
                                op0=mybir.AluOpType.is_equal,
                                op1=mybir.AluOpType.mult)
                            nc.tensor.matmul(
                                out=ps[:, sub * 128:(sub + 1) * 128],
                                lhsT=shb[:, t_idx * F:(t_idx + 1) * F],
                                rhs=Sd[:, :128],
                                start=(True if no_chunks else False),
                                stop=last, skip_group_check=True)
                            continue
                        cls, cid = op[1]
                        b, slab = divmod(cid, BCH)
                        if b != issued[cls]:
                            nb = min(BCH, nch_cls[cls] - b * BCH)
                            mt = msgp.tile([128, BCH, F], f16, tag=f"msg{cls}")
                            if variant == "memset":
                                nc.vector.memset(mt[:, :nb, :], 0.0)
                            else:
                                qn = ((gctr[0] if gcfg["qg"] else b)
                                      % gcfg["nq"])
                                gctr[0] += 1
                                nc.gpsimd.dma_gather(
                                    mt[:, :nb, :], tbl_ap[cls],
                                    idx_t[cls][:, b * (BCH * 8):
                                               b * (BCH * 8) + nb * 8],
                                    nb * CH, nb * CH, F,
                                    single_packet=gcfg["sp"],
                                    queue_num=qn)
                            issued[cls] = b
                            cur[cls] = mt
                        if variant == "gatheronly":
                            g += 1
                            continue
                        S = sp.tile([128, WINW], f16, tag="S")
                        nc.vector.tensor_scalar(
                            out=S[:, :width], in0=iota_t[:, :width],
                            scalar1=dl32[:, g:g + 1],
                            scalar2=nr32[:, g:g + 1],
                            op0=mybir.AluOpType.is_equal,
                            op1=mybir.AluOpType.mult)
                        if variant == "gs":
                            g += 1
                            continue
                        nc.tensor.matmul(
                            out=ps[:, :width], lhsT=cur[cls][:, slab, :],
                            rhs=S[:, :width],
                            start=first, stop=last)
                        g += 1
                    if variant in ("gatheronly", "gs"):
                        continue
                    for sub in range(width // 128):
                        t_idx = w * (WINW // 128) + sub
                        pslice = ps[:, sub * 128:(sub + 1) * 128]
                        dst_sl = shb[:, t_idx * F:(t_idx + 1) * F]
                        if l < 2:
                            # relu(agg + b_l) in feature-major, then @W_{l+1}
                            # transposes to node-major for the next table
                            aggT = aggp.tile([128, 128], f16, tag="aggT")
                            nc.scalar.activation(
                                out=aggT[:], in_=pslice, func=relu_fn,
                                bias=bcols3_t[:, l:l + 1])
                            if variant == "gsm":
                                continue
                            tp = hps.tile([128, F], f32, tag="hp")
                            nc.tensor.matmul(out=tp[:], lhsT=aggT[:],
                                             rhs=w_t[l + 1][:],
                                             start=True, stop=True)
                            nc.scalar.copy(out=dst_sl, in_=tp[:])
                        else:
                            # H3^T = agg + b2 (no relu), transpose to
                            # node-major for pooling
                            aggT = aggp.tile([128, 128], f16, tag="aggT")
                            nc.scalar.add(out=aggT[:], in_=pslice,
                                          add=bcols3_t[:, 2:3])
                            if variant == "gsm":
                                continue
                            tp = hps.tile([128, 128], f16, tag="hp",
                                          name="tppose")
                            nc.tensor.transpose(out=tp[:], in_=aggT[:],
                                                identity=ident[:])
                            nc.scalar.copy(out=dst_sl, in_=tp[:])
                assert g == ncht
                if variant in ("gatheronly", "gs", "gsm"):
                    continue
                if l < 2:
                    nc.sync.dma_start(
                        shard_d.ap().rearrange("(t p) f -> p t f", p=128),
                        shb[:, :].rearrange("p (t f) -> p t f", f=F))
                    if variant not in ("nocc",):
                        nc.gpsimd.collective_compute(
                            "AllGather", mybir.AluOpType.bypass,
                            replica_groups=[list(range(NCORES))],
                            ins=[shard_d[:].opt()], outs=[tabs[l + 1][:].opt()])

            # ---- mean pool ----
            if variant in ("gatheronly", "gs", "gsm"):
                z = tmpp.tile([GS, NO], f32, tag="ot", name="zot")
                nc.vector.memset(z[:], 0.0)
                nc.sync.dma_start(out[:], z[:])
                raise _SkipRest
            gp = gps.tile([128, G], f32, tag="gp")
            for t in range(TILES):
                Gt = sp.tile([128, G], f16, tag="S")
                nc.vector.tensor_scalar(
                    out=Gt[:], in0=iota_t[:],
                    scalar1=bcol_t[:, t:t + 1], scalar2=invc_t[:, t:t + 1],
                    op0=mybir.AluOpType.is_equal, op1=mybir.AluOpType.mult)
                nc.tensor.matmul(out=gp[:], lhsT=shb[:, t * F:(t + 1) * F],
                                 rhs=Gt[:], start=(t == 0), stop=(t == TILES - 1))
            gtile = tmpp.tile([128, G], f32, tag="gtile")
            nc.scalar.copy(out=gtile[:], in_=gp[:])
            nc.sync.dma_start(gt_in[:], gtile[:])
            if variant not in ("nocc",):
                nc.gpsimd.collective_compute(
                    "AllReduce", mybir.AluOpType.add,
                    replica_groups=[list(range(NCORES))],
                    ins=[gt_in[:].opt()], outs=[gt_out[:].opt()])
            gfull = tmpp.tile([128, G], f32, tag="gfull")
            nc.sync.dma_start(gfull[:], gt_out[:])
            gt16 = tmpp.tile([128, G], f16, tag="gt16")
            nc.vector.tensor_copy(gt16[:], gfull[:])

            # ---- MLP (hidden dim sharded; partials reduce-scattered) ----
            mp = mps.tile([NHS, G], f32, tag="mp")
            nc.tensor.matmul(out=mp[:], lhsT=wm1s_t[:], rhs=gt16[:],
                             start=True, stop=True)
            mt16 = tmpp.tile([NHS, G], f16, tag="mt16")
            nc.scalar.activation(out=mt16[:], in_=mp[:], func=relu_fn,
                                 bias=bm1s_t[:])
            for gh in range(G // 128):
                op = mps.tile([128, NO], f32, tag="op")
                nc.tensor.matmul(
                    out=op[:], lhsT=mt16[:, 128 * gh:128 * (gh + 1)],
                    rhs=wm2s_t[:], start=True, stop=False)
                nc.tensor.matmul(
                    out=op[:], lhsT=ones1[:], rhs=bm2s8_t[:],
                    start=False, stop=True)
                ot = tmpp.tile([128, NO], f32, tag="ot")
                nc.scalar.copy(out=ot[:], in_=op[:])
                nc.sync.dma_start(part_d[128 * gh:128 * (gh + 1), :], ot[:])
            if variant not in ("nocc",):
                nc.gpsimd.collective_compute(
                    "ReduceScatter", mybir.AluOpType.add,
                    replica_groups=[list(range(NCORES))],
                    ins=[part_d[:].opt()], outs=[rs_out[:].opt()])
            rt = tmpp.tile([GS, NO], f32, tag="rt")
            nc.sync.dma_start(rt[:], rs_out[:])
            nc.sync.dma_start(out[:], rt[:])

    nc.compile()
    return nc


def _get_built(inputs, variant="full"):
    import hashlib
    h = hashlib.sha1()
    h.update(np.ascontiguousarray(inputs["edge_index"]).tobytes())
    h.update(np.ascontiguousarray(inputs["batch"]).tobytes())
    key = (variant,
           tuple(sorted((k, v.shape, str(v.dtype)) for k, v in inputs.items())),
           h.hexdigest())
    if key not in _cache:
        geom, in_maps = _host_prep(**inputs)
        nc = _build_bass(geom, variant)
        _cache[key] = (geom, nc)
    else:
        geom, nc = _cache[key]
        _, in_maps = _host_prep(**inputs)
    return geom, nc, in_maps


def kernel(**inputs):
    inputs = {k: np.asarray(v) for k, v in inputs.items()}
    geom, nc, in_maps = _get_built(inputs)
    from concourse.bass_utils import run_bass_kernel_spmd
    res = run_bass_kernel_spmd(nc, in_maps, list(range(NCORES)))
    return np.concatenate([np.asarray(res.results[c]["out"])
                           for c in range(NCORES)], axis=0)
